# revision 82
# baseline (speedup 1.0000x reference)
"""Trainium2 Bass kernel for nn_Attention_15857019256917 (ViTDet-style attention
with decomposed relative position bias).

Sharding: data-parallel over B (2) x head-parallel (12 heads -> 4 groups of 3)
= 8 cores. Each core computes 3 heads of attention for one batch element plus
its partial output projection (rows of proj_w for its heads); the host sums the
4 partials per batch element (row-parallel linear unshard) and adds the bias
(with the v-bias folded in exactly: P@(V+1 bv^T)/l = PV/l + bv^T).

Default device algorithm (_emit_v4, rel err ~3.5e-3 on HW vs the 2e-2 gate):
  - q/k/v projections in bf16 (host-prepped bf16 x and weights), full
    contraction accumulated in PSUM, evacuated once (DVE lo-half + ACT
    hi-half with the bias fused).
  - The rel_h bias is FUSED into the QK^T matmul: contraction rows 64:112
    carry a static one-hot Eh (stationary, k side) against the per-head
    rel_h table (moving, q side) that the rel-table phase writes into
    qT[64:112].  rel_w is a second one-hot matmul (48-row contraction)
    against the per-head rel_w table.  All logit-path matmuls are f32r
    (1 cyc/row at >=256-wide moving vs fp32's 4).
  - P = exp(S) in bf16; PV in bf16 with a ones-row per head producing the
    softmax denominators in PSUM row 64.  The PV matmul is software-
    pipelined two tiles behind S so the PE never waits on the ACT exp.
  - psO evacuation fuses the softmax normalization: outT = psO[0:64] *
    gpsimd-partition-broadcast(1/psO[64]).
  - Output projection accumulates all 3 heads into one PSUM group per
    token tile; output DMAs alternate HWDGE queues.
  BIR f32r rule honored throughout: every f32r matmul operand is produced
  rounded -- ACT/DVE writes with f32r out dtype, or DRAM tensors DECLARED
  f32r carrying host-pre-rounded (bf16-exact) bits so plain sync DMAs are
  cast-free.  No gpsimd casting DMAs remain (Q7-software DMA processing
  measured far slower on HW than the cost model).  f32r with a strided
  moving operand fails the walrus ISA check (the 48-wide rel-table
  matmuls use bf16 copies of q instead).

KERNEL_SAFE=1 falls back to the original fp32/f32r-split emit (_emit).
"""
import os
import sys

sys.path.insert(0, "/opt/trn_rl_repo")
# a previously-crashed process can leave a NeuronCore in
# NRT_EXEC_UNIT_UNRECOVERABLE; resetting on open recovers it
os.environ.setdefault("NEURON_RT_RESET_CORES", "1")

import numpy as np

import concourse.bass as bass
import concourse.bacc as bacc
import concourse.tile as tile
from concourse import mybir
from concourse.masks import make_identity

F32 = mybir.dt.float32
F32R = mybir.dt.float32r
ACTF = mybir.ActivationFunctionType

B, H, W, D = 2, 48, 48, 768
NH, HD = 12, 64
S = H * W                      # 2304
SCALE = HD ** -0.5
N_CORES = 8
NHC = 3                        # heads per core
KT = S // 128                  # 18 key tiles
TOKT = S // 128                # 18 token tiles
KCH = D // 128                 # 6 contraction chunks
QT = [(0, 512), (512, 512), (1024, 512), (1536, 512), (2048, 256)]
VST = NHC * (HD + 1)           # 195: per-ktile V layout [v_h0|1|v_h1|1|v_h2|1]
WAVES = [(0, 2), (2, 4), (4, 6)]   # xT chunk waves (chunks [lo, hi))


def _ap(t, off_elems, dims):
    """Raw AP on tile t: partition dim copied, free dims = [[step, count], ...]."""
    return bass.AP(tensor=t.tensor, offset=t.offset + off_elems, ap=[t.ap[0]] + dims)


def _emit(tc, nc, aps, pfx="", p_split=True, bias_split=True, fast=False,
          stop_after="full", dbg=None):
    xT, wqk, bqk, wv, wp, RhT, RwT, Ecomb, zeros16, y = aps
    from contextlib import ExitStack

    if fast:
        # single-pass f32r everywhere; no hi/lo splits (rel err ~1e-3 << 2e-2)
        p_split = bias_split = False
    MMT = F32R if fast else F32      # dtype for matmul operand tiles

    def _r(ap):
        # bitcast an AP to f32r at matmul use sites (bit-identical data)
        return ap.bitcast(F32R) if fast else ap

    with ExitStack() as es:
        consts = es.enter_context(tc.tile_pool(name=pfx + "consts", bufs=1))
        big = es.enter_context(tc.tile_pool(name=pfx + "big", bufs=1))

        RhT_sb = consts.tile([HD, S], F32)
        nc.sync.dma_start(out=RhT_sb, in_=RhT)
        RwT_sb = consts.tile([HD, S], F32)
        nc.sync.dma_start(out=RwT_sb, in_=RwT)
        Ec_sb = consts.tile([112, S], F32R)
        nc.gpsimd.dma_start(out=Ec_sb, in_=Ecomb)

        qT = big.tile([128, NHC * S], F32)
        kT = big.tile([128, NHC * S], F32)
        outT = [big.tile([HD + 1, S], F32, name=f"outT{j}", tag=f"outT{j}")
                for j in range(NHC)]
        reciplc = big.tile([128, NHC * TOKT], F32)
        # V (with interleaved ones columns), f32r hi/lo split when p_split
        if p_split:
            v_hi = big.tile([128, TOKT * VST], F32R, name="v_hi", tag="v_hi")
            v_lo = big.tile([128, TOKT * VST], F32R, name="v_lo", tag="v_lo")
        else:
            v_hi = big.tile([128, TOKT * VST], F32R if fast else F32,
                            name="v_hi", tag="v_hi")
            v_lo = None

        # ---------------- phase 1: qkv projections ----------------
        with tc.tile_pool(name=pfx + "ph1", bufs=1) as ph1, \
             tc.tile_pool(name=pfx + "xw", bufs=2) as xw, \
             tc.tile_pool(name=pfx + "ps_qk", bufs=2, space="PSUM") as ps_qk, \
             tc.tile_pool(name=pfx + "ps_v", bufs=2, space="PSUM") as ps_v:
            wqk_sb = ph1.tile([128, KCH * 2 * NHC * HD], MMT)   # [128, 6*384]
            wv_sb = ph1.tile([128, KCH * NHC * HD], MMT)        # [128, 6*192]
            bqk_sb = ph1.tile([128, NHC], F32)                  # half-stacked biases
            nc.sync.dma_start(out=bqk_sb, in_=bqk)
            v32 = (ph1.tile([128, TOKT * VST], F32, name="v32")
                   if (p_split or fast) else v_hi)
            nc.vector.memset(_ap(v32, HD, [[VST, TOKT], [HD + 1, NHC]]), 1.0)
            wqk_dst = wqk_sb.bitcast(F32) if fast else wqk_sb
            wv_dst = wv_sb.bitcast(F32) if fast else wv_sb
            for k in range(KCH):
                nc.sync.dma_start(out=wqk_dst[:, k * 384:(k + 1) * 384],
                                  in_=wqk[k * 128:(k + 1) * 128, :])
                nc.sync.dma_start(out=wv_dst[:, k * 192:(k + 1) * 192],
                                  in_=wv[k * 128:(k + 1) * 128, :])

            # M-tiles (128 rows = two 64-channel halves):
            #   T0=[q0|q1]  T1=[q2|k0]  T2=[k1|k2]
            # low halves copy straight to rows 0-63 of their dest tensor; high
            # halves park in the dest tensor's padding rows 64-127 (same column
            # range), then an intra-tensor DMA partition-shifts them down.
            lo_dest = [(qT, 0), (qT, 2), (kT, 1)]
            hi_dest = [(qT, 1), (kT, 0), (kT, 2)]
            for wave, (klo, khi) in enumerate(WAVES):
                xs = []
                for k in range(klo, khi):
                    xt = xw.tile([128, S], MMT, name=f"x{k}", tag="x")
                    nc.sync.dma_start(out=xt.bitcast(F32) if fast else xt,
                                      in_=xT[k * 128:(k + 1) * 128, :])
                    xs.append(xt)
                for m in range(NHC):
                    for (n0, nw) in QT:
                        ps = ps_qk.tile([128, 512], F32, tag="qk")
                        for i, k in enumerate(range(klo, khi)):
                            nc.tensor.matmul(
                                ps[:, :nw],
                                wqk_sb[:, k * 384 + m * 128: k * 384 + (m + 1) * 128],
                                xs[i][:, n0:n0 + nw],
                                start=(i == 0), stop=(i == khi - klo - 1))
                        lt_, lh = lo_dest[m]
                        ht_, hh = hi_dest[m]
                        dlo = lt_[0:64, lh * S + n0: lh * S + n0 + nw]
                        dhi = ht_[64:128, hh * S + n0: hh * S + n0 + nw]
                        if wave == 0:
                            nc.scalar.activation(out=dlo, in_=ps[0:64, :nw],
                                                 func=ACTF.Identity,
                                                 bias=bqk_sb[0:64, m:m + 1])
                            nc.scalar.activation(out=dhi, in_=ps[64:128, :nw],
                                                 func=ACTF.Identity,
                                                 bias=bqk_sb[64:128, m:m + 1])
                        else:
                            nc.vector.tensor_add(dlo, dlo, ps[0:64, :nw])
                            nc.vector.tensor_add(dhi, dhi, ps[64:128, :nw])
                # V natural layout
                for ts in range(TOKT):
                    ps = ps_v.tile([128, NHC * HD], F32, tag="v")
                    for i, k in enumerate(range(klo, khi)):
                        nc.tensor.matmul(
                            ps[:],
                            xs[i][:, ts * 128:(ts + 1) * 128],
                            wv_sb[:, k * 192:(k + 1) * 192],
                            start=(i == 0), stop=(i == khi - klo - 1))
                    vdst = _ap(v32, ts * VST, [[HD + 1, NHC], [1, HD]])
                    vsrc = _ap(ps, 0, [[HD, NHC], [1, HD]])
                    if wave == 0:
                        nc.scalar.activation(out=vdst, in_=vsrc, func=ACTF.Copy)
                    else:
                        nc.vector.tensor_add(vdst, vdst, vsrc)
            # partition-shift the parked high halves into place
            for m in range(NHC):
                ht_, hh = hi_dest[m]
                nc.sync.dma_start(out=ht_[0:64, hh * S:(hh + 1) * S],
                                  in_=ht_[64:128, hh * S:(hh + 1) * S])
            # split V into f32r hi + lo (ones cols stay exact: 1.0 and 0.0)
            if p_split:
                nc.scalar.activation(out=v_hi, in_=v32, func=ACTF.Copy)
                nc.vector.tensor_sub(v_lo, v32, v_hi.bitcast(F32))
            elif fast:
                nc.scalar.activation(out=v_hi, in_=v32, func=ACTF.Copy)

        if stop_after == "qkv":
            nc.sync.dma_start(out=dbg["qT"], in_=qT)
            nc.sync.dma_start(out=dbg["kT"], in_=kT)
            nc.sync.dma_start(out=dbg["v"],
                              in_=v_hi.bitcast(F32) if p_split else v_hi)
            return

        late = es.enter_context(tc.tile_pool(name=pfx + "late", bufs=1))
        wp_sb = []
        for j in range(NHC):
            t = late.tile([HD, D], MMT, name=f"wp{j}", tag=f"wp{j}")
            nc.sync.dma_start(out=t.bitcast(F32) if fast else t, in_=wp[j])
            wp_sb.append(t)
        ident = late.tile([128, 128], F32)
        make_identity(nc, ident)
        # bias tables: rows 0-47 rel_w, 48-63 zero, 64-111 rel_h; hi/lo split
        relT = late.tile([112, S], F32R, name="relT", tag="relT")
        nc.gpsimd.dma_start(out=relT[48:64, :], in_=zeros16)
        relTlo = None
        if bias_split:
            relTlo = late.tile([112, S], F32R, name="relTlo", tag="relTlo")
            nc.gpsimd.dma_start(out=relTlo[48:64, :], in_=zeros16)

        # ---------------- phases 2+3: per-head attention ----------------
        with tc.tile_pool(name=pfx + "rel32p", bufs=1) as rel32p, \
             tc.tile_pool(name=pfx + "pTp", bufs=3) as pTp, \
             tc.tile_pool(name=pfx + "lp", bufs=2) as lp, \
             tc.tile_pool(name=pfx + "ps_rel", bufs=2, space="PSUM") as ps_rel, \
             tc.tile_pool(name=pfx + "ps_S", bufs=2, space="PSUM") as ps_S, \
             tc.tile_pool(name=pfx + "ps_O", bufs=2, space="PSUM") as ps_O:
            rel32 = (rel32p.tile([112, S], F32, name="rel32")
                     if bias_split else None)
            for h in range(NHC):
                # rel tables: batches of 10 row-indices share one psum bank;
                # each bank gets exactly two accumulation groups (rel_w rows
                # 0-47 and rel_h rows 64-111, disjoint partitions)
                rel_dst = rel32 if bias_split else relT
                for g in range(5):
                    cnt = 10 if g < 4 else 8
                    ps = ps_rel.tile([128, 480], F32, tag="rel")
                    for i in range(cnt):
                        r = g * 10 + i
                        nc.tensor.matmul(
                            ps[0:48, i * 48:(i + 1) * 48],
                            _r(RwT_sb[:, r * 48:(r + 1) * 48]),
                            _r(bass.AP(tensor=qT.tensor,
                                       offset=qT.offset + h * S + r,
                                       ap=[qT[0:64, :].ap[0], [48, 48]])),
                            start=(i == 0), stop=(i == cnt - 1))
                        # out at base partition 64 (col-tiled); the sim's
                        # zero-region bookkeeping mis-indexes partition-offset
                        # psum APs, so skip its group check (single writer per
                        # element; overwrite-vs-accumulate equivalent here)
                        nc.tensor.matmul(
                            ps[64:112, i * 48:(i + 1) * 48],
                            _r(RhT_sb[:, r * 48:(r + 1) * 48]),
                            _r(qT[0:64, h * S + r * 48: h * S + (r + 1) * 48]),
                            start=(i == 0), stop=(i == cnt - 1),
                            skip_group_check=True)
                    nc.scalar.activation(
                        out=rel_dst[64:112, g * 480: g * 480 + cnt * 48],
                        in_=ps[64:112, 0:cnt * 48], func=ACTF.Copy)
                    wdst = bass.AP(tensor=rel_dst.tensor,
                                   offset=rel_dst.offset + g * 10,
                                   ap=[rel_dst[0:48, :].ap[0], [1, cnt], [48, 48]])
                    wsrc = bass.AP(tensor=ps.tensor, offset=ps.offset,
                                   ap=[ps[0:48, :].ap[0], [48, cnt], [1, 48]])
                    nc.scalar.activation(out=wdst, in_=wsrc, func=ACTF.Copy)
                if bias_split:
                    # hi/lo split (rows 0-47 and 64-111; zero rows preset)
                    for r0, r1 in [(0, 48), (64, 112)]:
                        nc.scalar.activation(out=relT[r0:r1, :],
                                             in_=rel32[r0:r1, :], func=ACTF.Copy)
                        nc.vector.tensor_sub(relTlo[r0:r1, :], rel32[r0:r1, :],
                                             relT[r0:r1, :].bitcast(F32))

                if stop_after == "rel":
                    nc.gpsimd.dma_start(out=dbg["relT"], in_=relT)
                    return

                # attention
                for (q0, qw) in QT:
                    psO = ps_O.tile([HD + 1, 512], F32, tag="o")
                    for kt in range(KT):
                        psS = ps_S.tile([128, 512], F32, tag="s")
                        nc.tensor.matmul(
                            psS[:, :qw],
                            _r(kT[0:64, h * S + kt * 128: h * S + (kt + 1) * 128]),
                            _r(qT[0:64, h * S + q0: h * S + q0 + qw]),
                            start=True, stop=False)
                        nc.tensor.matmul(
                            psS[:, :qw],
                            Ec_sb[:, kt * 128:(kt + 1) * 128],
                            relT[:, q0:q0 + qw],
                            start=False, stop=not bias_split)
                        if bias_split:
                            nc.tensor.matmul(
                                psS[:, :qw],
                                Ec_sb[:, kt * 128:(kt + 1) * 128],
                                relTlo[:, q0:q0 + qw],
                                start=False, stop=True)
                        pT = pTp.tile([128, 512],
                                      F32R if (p_split or fast) else F32,
                                      tag="p")
                        nc.scalar.activation(out=pT[:, :qw], in_=psS[:, :qw],
                                             func=ACTF.Exp)
                        vsl = slice(kt * VST + h * (HD + 1),
                                    kt * VST + (h + 1) * (HD + 1))
                        nc.tensor.matmul(
                            psO[:, :qw], v_hi[:, vsl], pT[:, :qw],
                            start=(kt == 0),
                            stop=(kt == KT - 1 and not p_split))
                        if p_split:
                            nc.tensor.matmul(
                                psO[:, :qw], v_lo[:, vsl], pT[:, :qw],
                                start=False, stop=(kt == KT - 1))
                    nc.scalar.activation(out=outT[h][:, q0:q0 + qw],
                                         in_=psO[:, :qw], func=ACTF.Copy)

                # softmax denominators -> per-token columns, reciprocal
                psT = ps_O.tile([128, TOKT], F32, tag="t", bufs=2)
                for ts in range(TOKT):
                    nc.tensor.matmul(psT[:, ts:ts + 1],
                                     outT[h][HD:HD + 1, ts * 128:(ts + 1) * 128],
                                     ident[HD:HD + 1, HD:HD + 1],
                                     is_transpose=True,
                                     start=(ts == 0), stop=(ts == TOKT - 1))
                lcols = lp.tile([128, TOKT], F32, tag="lc")
                nc.scalar.activation(out=lcols, in_=psT, func=ACTF.Copy)
                nc.vector.reciprocal(out=reciplc[:, h * TOKT:(h + 1) * TOKT],
                                     in_=lcols)
                if stop_after == "attn1":
                    nc.sync.dma_start(out=dbg["outT"], in_=outT[0])
                    nc.sync.dma_start(out=dbg["recip"], in_=reciplc)
                    return

        if stop_after == "attn3":
            return

        # ---------------- phase 4: output projection ----------------
        with tc.tile_pool(name=pfx + "yw", bufs=2) as yw, \
             tc.tile_pool(name=pfx + "ps_y", bufs=2, space="PSUM") as ps_y:
            for ts in range(TOKT):
                y_acc = yw.tile([128, D], F32, tag="yacc")
                for h in range(NHC):
                    ps = ps_y.tile([128, D], F32, tag="y")
                    for (n0, nw) in [(0, 512), (512, 256)]:
                        nc.tensor.matmul(ps[:, n0:n0 + nw],
                                         _r(outT[h][0:HD, ts * 128:(ts + 1) * 128]),
                                         wp_sb[h][:, n0:n0 + nw],
                                         start=True, stop=True)
                    scal = reciplc[:, h * TOKT + ts: h * TOKT + ts + 1]
                    if h == 0:
                        nc.vector.tensor_scalar_mul(out=y_acc, in0=ps[:],
                                                    scalar1=scal)
                    else:
                        z = yw.tile([128, D], F32, tag="ztmp", bufs=1)
                        nc.vector.tensor_scalar_mul(out=z, in0=ps[:], scalar1=scal)
                        nc.vector.tensor_add(y_acc, y_acc, z)
                nc.sync.dma_start(out=y[ts * 128:(ts + 1) * 128, :], in_=y_acc)


def _emit_v4(tc, nc, aps, pfx=""):
    """Optimized emit: f32r/bf16 matmuls (1 cyc/row vs fp32's 4), rel_h bias
    fused into the QK^T matmul via contraction rows 64:112 (one-hot Eh
    stationary x relh-table moving), rel_w bias via DVE adds from 3
    phase-replicated tables, per-column 1/l scaling of outT so the output
    projection accumulates all heads in one PSUM group.

    BIR constraint honored throughout: every f32r matmul operand is produced
    by a rounding instruction (ACT/DVE out=f32r, or gpsimd casting DMA).
    """
    xT, wqk, bqk, wv, wp, RhT, RwT, Ew48, EhT, y = aps
    from contextlib import ExitStack
    BF16 = mybir.dt.bfloat16

    with ExitStack() as es:
        consts = es.enter_context(tc.tile_pool(name=pfx + "consts", bufs=1))
        big = es.enter_context(tc.tile_pool(name=pfx + "big", bufs=1))

        RhT_sb = consts.tile([HD, S], BF16)
        nc.scalar.dma_start(out=RhT_sb, in_=RhT)     # host-prepped bf16
        RwT_sb = consts.tile([HD, S], BF16)
        nc.scalar.dma_start(out=RwT_sb, in_=RwT)
        Ec48 = consts.tile([48, S], BF16)            # rel_w one-hot (w2(k))
        nc.sync.dma_start(out=Ec48, in_=Ew48)        # host-prepped bf16

        # qT/kT rows 0:64 = q/k channels; qT rows 64:112 get the per-head
        # relh table, kT rows 64:112 the static Eh one-hot (fused rel_h)
        qT = big.tile([128, NHC * S], F32R)
        kT = big.tile([128, NHC * S], F32R)
        outT = [big.tile([HD, S], F32R, name=f"outT{j}", tag=f"outT{j}")
                for j in range(NHC)]
        v_hi = big.tile([128, TOKT * VST], BF16, name="v_hi", tag="v_hi")
        # bf16 copies of q for the 48-wide rel-table matmuls; filled inside
        # phase 1 as soon as each head's q halves are complete so the rel
        # phase of head 0 starts without waiting on the full projection
        qbf = [big.tile([HD, S], BF16, name=f"qbf{j}", tag=f"qbf{j}")
               for j in range(NHC)]

        # ---------------- phase 1: qkv projections (bf16) ----------------
        with tc.tile_pool(name=pfx + "ph1", bufs=1) as ph1, \
             tc.tile_pool(name=pfx + "xw", bufs=1) as xw, \
             tc.tile_pool(name=pfx + "ps_qk", bufs=2, space="PSUM") as ps_qk, \
             tc.tile_pool(name=pfx + "ps_v", bufs=2, space="PSUM") as ps_v:
            bqk_sb = ph1.tile([128, NHC], F32)
            nc.sync.dma_start(out=bqk_sb, in_=bqk)
            v32 = ph1.tile([128, TOKT * VST], F32, name="v32")
            nc.vector.memset(_ap(v32, HD, [[VST, TOKT], [HD + 1, NHC]]), 1.0)
            wqk_sb = ph1.tile([128, KCH * 2 * NHC * HD], BF16)
            wv_sb = ph1.tile([128, KCH * NHC * HD], BF16)
            # weights on the scalar HWDGE queue so they don't serialize
            # behind the x stream on the sync queue
            for k in range(KCH):
                nc.scalar.dma_start(out=wqk_sb[:, k * 384:(k + 1) * 384],
                                    in_=wqk[k * 128:(k + 1) * 128, :])
                nc.scalar.dma_start(out=wv_sb[:, k * 192:(k + 1) * 192],
                                    in_=wv[k * 128:(k + 1) * 128, :])

            lo_dest = [(qT, 0), (qT, 2), (kT, 1)]
            hi_dest = [(qT, 1), (kT, 0), (kT, 2)]
            # kT range 1 rows 64:112 are never used for hi-half parking, so
            # its Eh one-hot block can load immediately (EhT is declared f32r
            # in DRAM: plain dma, no gpsimd cast needed)
            nc.scalar.dma_start(out=kT[64:112, 1 * S:2 * S], in_=EhT)
            # all 6 x chunks resident in bf16 (3.4MB): full contraction in
            # PSUM, single evacuation per output tile
            xs = []
            for k in range(KCH):
                xt = xw.tile([128, S], BF16, name=f"x{k}", tag=f"x{k}")
                nc.sync.dma_start(out=xt, in_=xT[k * 128:(k + 1) * 128, :])
                xs.append(xt)
            for m in range(NHC):
                for (n0, nw) in QT:
                    ps = ps_qk.tile([128, 512], F32, tag="qk")
                    for k in range(KCH):
                        nc.tensor.matmul(
                            ps[:, :nw],
                            wqk_sb[:, k * 384 + m * 128: k * 384 + (m + 1) * 128],
                            xs[k][:, n0:n0 + nw],
                            start=(k == 0), stop=(k == KCH - 1))
                    lt_, lh = lo_dest[m]
                    ht_, hh = hi_dest[m]
                    dlo = lt_[0:64, lh * S + n0: lh * S + n0 + nw]
                    dhi = ht_[64:128, hh * S + n0: hh * S + n0 + nw]
                    nc.vector.tensor_scalar_add(dlo, ps[0:64, :nw],
                                                bqk_sb[0:64, m:m + 1])
                    nc.scalar.activation(out=dhi, in_=ps[64:128, :nw],
                                         func=ACTF.Identity,
                                         bias=bqk_sb[64:128, m:m + 1])
                # as soon as this m's halves are evacuated: shift the parked
                # hi half into place, load the freed Eh block, and emit any
                # qbf copy whose sources are now complete -- this lets head
                # 0's rel phase start while V is still projecting
                ht_, hh = hi_dest[m]
                nc.sync.dma_start(out=ht_[0:64, hh * S:(hh + 1) * S],
                                  in_=ht_[64:128, hh * S:(hh + 1) * S])
                if ht_ is kT:
                    nc.scalar.dma_start(out=kT[64:112, hh * S:(hh + 1) * S],
                                        in_=EhT)
                if m == 0:
                    nc.vector.tensor_copy(qbf[0], qT[0:64, 0:S].bitcast(F32))
                    nc.vector.tensor_copy(qbf[1],
                                          qT[0:64, S:2 * S].bitcast(F32))
                elif m == 1:
                    nc.vector.tensor_copy(qbf[2],
                                          qT[0:64, 2 * S:3 * S].bitcast(F32))
            # V natural layout (bf16 moving: 1 cyc/row even at width 192)
            for ts in range(TOKT):
                ps = ps_v.tile([128, NHC * HD], F32, tag="v")
                for k in range(KCH):
                    nc.tensor.matmul(
                        ps[:],
                        xs[k][:, ts * 128:(ts + 1) * 128],
                        wv_sb[:, k * 192:(k + 1) * 192],
                        start=(k == 0), stop=(k == KCH - 1))
                vdst = _ap(v32, ts * VST, [[HD + 1, NHC], [1, HD]])
                vsrc = _ap(ps, 0, [[HD, NHC], [1, HD]])
                nc.scalar.activation(out=vdst, in_=vsrc, func=ACTF.Copy)
            nc.scalar.activation(out=v_hi, in_=v32, func=ACTF.Copy)

        late = es.enter_context(tc.tile_pool(name=pfx + "late", bufs=1))
        wp_sb = []
        for j in range(NHC):
            t = late.tile([HD, D], F32R, name=f"wp{j}", tag=f"wp{j}")
            nc.scalar.dma_start(out=t, in_=wp[j])   # wpr: bf16-exact f32r
            wp_sb.append(t)

        # ---------------- phases 2+3: per-head attention ----------------
        with tc.tile_pool(name=pfx + "pTp", bufs=4) as pTp, \
             tc.tile_pool(name=pfx + "lp", bufs=2) as lp, \
             tc.tile_pool(name=pfx + "ps_rel", bufs=2, space="PSUM") as ps_rel, \
             tc.tile_pool(name=pfx + "ps_S", bufs=4, space="PSUM") as ps_S, \
             tc.tile_pool(name=pfx + "ps_O", bufs=2, space="PSUM") as ps_O:
            for h in range(NHC):
                # rel tables: rel_w rows -> relw tile; rel_h rows -> qT[64:112]
                # (relw double-buffered so head h+1's table build does not
                # wait behind head h's last rel_w matmul)
                relw = lp.tile([48, S], BF16, tag="relw")
                for g in range(5):
                    cnt = 10 if g < 4 else 8
                    ps = ps_rel.tile([128, 480], F32, tag="rel")
                    for i in range(cnt):
                        r = g * 10 + i
                        nc.tensor.matmul(
                            ps[0:48, i * 48:(i + 1) * 48],
                            RwT_sb[:, r * 48:(r + 1) * 48],
                            bass.AP(tensor=qbf[h].tensor,
                                    offset=qbf[h].offset + r,
                                    ap=[qbf[h][0:64, :].ap[0], [48, 48]]),
                            start=(i == 0), stop=(i == cnt - 1))
                        nc.tensor.matmul(
                            ps[64:112, i * 48:(i + 1) * 48],
                            RhT_sb[:, r * 48:(r + 1) * 48],
                            qbf[h][:, r * 48:(r + 1) * 48],
                            start=(i == 0), stop=(i == cnt - 1),
                            skip_group_check=True)
                    # rel-phase evacuations live on the ACT queue: its other
                    # work (exp) completes early, while the DVE queue blocks
                    # behind the previous head's late colscale ops
                    nc.scalar.activation(
                        out=qT[64:112, h * S + g * 480: h * S + g * 480 + cnt * 48],
                        in_=ps[64:112, 0:cnt * 48], func=ACTF.Copy)
                    wdst = bass.AP(tensor=relw.tensor,
                                   offset=relw.offset + g * 10,
                                   ap=[relw[0:48, :].ap[0], [1, cnt], [48, 48]])
                    wsrc = bass.AP(tensor=ps.tensor, offset=ps.offset,
                                   ap=[ps[0:48, :].ap[0], [48, cnt], [1, 48]])
                    # engine assignment is load-bearing (HW-measured):
                    # qT copy on ACT + this strided copy on DVE beat both
                    # on ACT (+37us) and both on DVE (+140us)
                    nc.vector.tensor_copy(wdst, wsrc)
                # attention: fused S = K^T q + rel_h (one-hot rows 64:112),
                # + rel_w via a second one-hot matmul, exp, PV.
                # PV is staggered one tile behind S/relw in PE program order
                # so the PE never stalls waiting for exp(t) on the ACT engine.
                for (q0, qw) in QT:
                    psO = ps_O.tile([HD + 1, 512], F32, tag="o")
                    pend = []
                    for kt in range(KT):
                        psS = ps_S.tile([128, 512], F32, tag="s")
                        nc.tensor.matmul(
                            psS[:, :qw],
                            kT[0:112, h * S + kt * 128: h * S + (kt + 1) * 128],
                            qT[0:112, h * S + q0: h * S + q0 + qw],
                            start=True, stop=False)
                        nc.tensor.matmul(
                            psS[:, :qw],
                            Ec48[:, kt * 128:(kt + 1) * 128],
                            relw[:, q0:q0 + qw],
                            start=False, stop=True)
                        if len(pend) >= 2:
                            p0 = pend.pop(0)
                            nc.tensor.matmul(
                                psO[:, :qw], v_hi[:, p0[1]], p0[0][:, :qw],
                                start=(p0[2] == 0), stop=False)
                        pT = pTp.tile([128, 512], BF16, tag="p")
                        nc.scalar.activation(out=pT[:, :qw], in_=psS[:, :qw],
                                             func=ACTF.Exp)
                        vsl = slice(kt * VST + h * (HD + 1),
                                    kt * VST + (h + 1) * (HD + 1))
                        pend.append((pT, vsl, kt))
                    for j, p0 in enumerate(pend):
                        nc.tensor.matmul(
                            psO[:, :qw], v_hi[:, p0[1]], p0[0][:, :qw],
                            start=(p0[2] == 0), stop=(j == len(pend) - 1))
                    # evacuate with the softmax normalization fused in:
                    # outT = psO[0:HD] * broadcast(1 / psO[row HD])
                    rec1 = lp.tile([1, 512], F32, tag="rec1")
                    # NOTE: reciprocal_approx_fast returns garbage on this
                    # HW (sim is fine) -- keep the exact reciprocal
                    nc.vector.reciprocal(out=rec1[:, :qw],
                                         in_=psO[HD:HD + 1, :qw])
                    rb = lp.tile([HD, 512], F32, tag="rb")
                    nc.gpsimd.partition_broadcast(rb[:, :qw], rec1[:, :qw])
                    nc.vector.tensor_mul(outT[h][0:HD, q0:q0 + qw],
                                         psO[0:HD, :qw], rb[:, :qw])

        # ---------------- phase 4: output projection ----------------
        with tc.tile_pool(name=pfx + "yw", bufs=3) as yw, \
             tc.tile_pool(name=pfx + "ps_y", bufs=3, space="PSUM") as ps_y:
            for ts in range(TOKT):
                ps = ps_y.tile([128, D], F32, tag="y")
                for h in range(NHC):
                    for (n0, nw) in [(0, 512), (512, 256)]:
                        nc.tensor.matmul(ps[:, n0:n0 + nw],
                                         outT[h][0:HD, ts * 128:(ts + 1) * 128],
                                         wp_sb[h][:, n0:n0 + nw],
                                         start=(h == 0), stop=(h == NHC - 1))
                y_acc = yw.tile([128, D], F32, tag="yacc")
                nc.vector.tensor_copy(y_acc, ps)
                # alternate HWDGE queues so the output stream drains ~2x
                # faster and the final-tile tail shrinks
                yq = nc.sync if ts % 2 == 0 else nc.scalar
                yq.dma_start(out=y[ts * 128:(ts + 1) * 128, :], in_=y_acc)


def build_nc(num_devices=N_CORES, p_split=True, bias_split=True, fast=False,
             stop_after="full", reps=1):
    nc = bacc.Bacc("TRN2", target_bir_lowering=False, debug=False,
                   num_devices=num_devices)
    BF16 = mybir.dt.bfloat16
    y_ap = nc.dram_tensor("y", [S, D], F32, kind="ExternalOutput").ap()
    if fast:
        aps_v4 = (
            nc.dram_tensor("xTbf", [D, S], BF16, kind="ExternalInput").ap(),
            nc.dram_tensor("wqkbf", [D, 2 * NHC * HD], BF16,
                           kind="ExternalInput").ap(),
            nc.dram_tensor("bqk", [128, NHC], F32, kind="ExternalInput").ap(),
            nc.dram_tensor("wvbf", [D, NHC * HD], BF16,
                           kind="ExternalInput").ap(),
            nc.dram_tensor("wpr", [NHC, HD, D], F32R,
                           kind="ExternalInput").ap(),
            nc.dram_tensor("RhTbf", [HD, S], BF16, kind="ExternalInput").ap(),
            nc.dram_tensor("RwTbf", [HD, S], BF16, kind="ExternalInput").ap(),
            nc.dram_tensor("Ew48", [48, S], BF16, kind="ExternalInput").ap(),
            nc.dram_tensor("EhTr", [48, S], F32R, kind="ExternalInput").ap(),
            y_ap,
        )
        aps = None
    else:
        aps = (
            nc.dram_tensor("xT", [D, S], F32, kind="ExternalInput").ap(),
            nc.dram_tensor("wqk", [D, 2 * NHC * HD], F32,
                           kind="ExternalInput").ap(),
            nc.dram_tensor("bqk", [128, NHC], F32, kind="ExternalInput").ap(),
            nc.dram_tensor("wv", [D, NHC * HD], F32, kind="ExternalInput").ap(),
            nc.dram_tensor("wp", [NHC, HD, D], F32, kind="ExternalInput").ap(),
            nc.dram_tensor("RhT", [HD, S], F32, kind="ExternalInput").ap(),
            nc.dram_tensor("RwT", [HD, S], F32, kind="ExternalInput").ap(),
            nc.dram_tensor("Ecomb", [112, S], F32, kind="ExternalInput").ap(),
            nc.dram_tensor("zeros16", [16, S], F32, kind="ExternalInput").ap(),
            y_ap,
        )
        aps_v4 = None
    dbg = {}
    if stop_after == "qkv":
        dbg["qT"] = nc.dram_tensor("dbg_qT", [HD, NHC * S], F32,
                                   kind="ExternalOutput").ap()
        dbg["kT"] = nc.dram_tensor("dbg_kT", [HD, NHC * S], F32,
                                   kind="ExternalOutput").ap()
        dbg["v"] = nc.dram_tensor("dbg_v", [128, TOKT * VST], F32,
                                  kind="ExternalOutput").ap()
    elif stop_after == "rel":
        dbg["relT"] = nc.dram_tensor("dbg_relT", [112, S], F32,
                                     kind="ExternalOutput").ap()
    elif stop_after == "attn1":
        dbg["outT"] = nc.dram_tensor("dbg_outT", [HD + 1, S], F32,
                                     kind="ExternalOutput").ap()
        dbg["recip"] = nc.dram_tensor("dbg_recip", [128, NHC * TOKT], F32,
                                      kind="ExternalOutput").ap()
    with tile.TileContext(nc) as tc:
        for rep in range(reps):
            pfx = f"r{rep}_" if reps > 1 else ""
            if fast:
                _emit_v4(tc, nc, aps_v4, pfx=pfx)
            else:
                _emit(tc, nc, aps, pfx=pfx,
                      p_split=p_split, bias_split=bias_split, fast=False,
                      stop_after=stop_after, dbg=dbg)
    nc.compile()
    return nc


def prep_core_inputs(c, x, qkv_w, qkv_b, proj_w, rel_pos_h, rel_pos_w):
    b = c // 4
    heads = [3 * (c % 4) + j for j in range(NHC)]
    f32 = np.float32
    xT = np.ascontiguousarray(np.asarray(x, f32)[b].reshape(S, D).T)
    qkv_w = np.asarray(qkv_w, f32)
    qkv_b = np.asarray(qkv_b, f32)
    wq = np.concatenate([qkv_w[:, h * HD:(h + 1) * HD] for h in heads], 1) * f32(SCALE)
    wk = np.concatenate([qkv_w[:, D + h * HD:D + (h + 1) * HD] for h in heads], 1)
    wqk = np.ascontiguousarray(np.concatenate([wq, wk], 1))
    bq = [qkv_b[h * HD:(h + 1) * HD] * f32(SCALE) for h in heads]
    bk = [qkv_b[D + h * HD:D + (h + 1) * HD] for h in heads]
    # per-M-tile half-stacked biases: [q0|q1], [q2|k0], [k1|k2]
    halves = [bq[0], bq[1], bq[2], bk[0], bk[1], bk[2]]
    bqk = np.stack([np.concatenate([halves[2 * m], halves[2 * m + 1]])
                    for m in range(NHC)], 1).astype(f32)
    wv = np.ascontiguousarray(
        np.concatenate([qkv_w[:, 2 * D + h * HD:2 * D + (h + 1) * HD]
                        for h in heads], 1))
    wp = np.ascontiguousarray(
        np.stack([np.asarray(proj_w, f32)[h * HD:(h + 1) * HD, :]
                  for h in heads], 0))
    coords = np.arange(H)[:, None] - np.arange(H)[None, :] + (H - 1)
    Rh = np.asarray(rel_pos_h, f32)[coords]      # [hq, hk, c]
    Rw = np.asarray(rel_pos_w, f32)[coords]      # [wq, wk, c]
    # The reference builds the rel bias from the UNSCALED q; we fold `SCALE`
    # into wq/bq, so fold the exact inverse (8.0) into the rel tables.
    inv = f32(1.0 / SCALE)
    RhT = np.ascontiguousarray(np.transpose(Rh, (2, 0, 1)).reshape(HD, S)) * inv
    RwT = np.ascontiguousarray(np.transpose(Rw, (2, 0, 1)).reshape(HD, S)) * inv
    E = np.zeros((112, S), f32)
    kk = np.arange(S)
    E[kk % W, kk] = 1.0           # rel_w one-hot rows 0..47
    E[64 + kk // W, kk] = 1.0     # rel_h one-hot rows 64..111
    Eh = np.zeros((48, S), f32)
    Eh[kk // W, kk] = 1.0         # rel_h one-hot for the fused S matmul
    import ml_dtypes
    bf16 = ml_dtypes.bfloat16
    return {"xT": xT, "wqk": wqk, "bqk": bqk, "wv": wv, "wp": wp,
            "RhT": RhT, "RwT": RwT, "Ecomb": E,
            "zeros16": np.zeros((16, S), f32), "EhT": Eh,
            "xTbf": xT.astype(bf16), "wqkbf": wqk.astype(bf16),
            "wvbf": wv.astype(bf16), "Ew48": E[0:48].astype(bf16),
            "RhTbf": RhT.astype(bf16), "RwTbf": RwT.astype(bf16),
            # declared f32r in DRAM: must carry pre-rounded bits (bf16-exact
            # values survive any further f32r truncation unchanged)
            "wpr": wp.astype(bf16).astype(f32), "EhTr": Eh}


_NC_CACHE = {}


def _get_nc(**kw):
    key = str(sorted(kw.items()))
    if key not in _NC_CACHE:
        _NC_CACHE[key] = build_nc(**kw)
    return _NC_CACHE[key]


def gather_output(ys, qkv_b, proj_w, proj_b):
    f32 = np.float32
    bp_eff = (np.asarray(proj_b, f32)
              + np.asarray(qkv_b, f32)[2 * D:] @ np.asarray(proj_w, f32))
    out = np.empty((B, H, W, D), f32)
    for b in range(B):
        acc = ys[4 * b].copy()
        for j in range(1, 4):
            acc += ys[4 * b + j]
        acc += bp_eff
        out[b] = acc.reshape(H, W, D)
    return out


def kernel(x, qkv_w, qkv_b, proj_w, proj_b, rel_pos_h, rel_pos_w):
    import os
    from concourse.bass_utils import run_bass_kernel_spmd
    safe = os.environ.get("KERNEL_SAFE", "0") == "1"
    nc = _get_nc(p_split=safe, fast=not safe)
    in_maps = [prep_core_inputs(c, x, qkv_w, qkv_b, proj_w, rel_pos_h, rel_pos_w)
               for c in range(N_CORES)]
    res = run_bass_kernel_spmd(nc, in_maps, core_ids=list(range(N_CORES)))
    ys = [res.results[c]["y"] for c in range(N_CORES)]
    return gather_output(ys, qkv_b, proj_w, proj_b)

